# revision 1
# baseline (speedup 1.0000x reference)
"""Complex Gaussian splatter field kernel for 8 TRN2 NeuronCores.

Math: field[m] = sum_n exp(-0.5*mah(m,n)) * exp(-i*k*dist(m,n)) * cv[n]
  mah  = (q-p)^T R diag(1/s^2) R^T (q-p)   (per-pair Mahalanobis)
  dist = |q-p|

Implementation strategy:
  - Host: Morton-sort query points; per m-tile (512 queries) center coords,
    build 10-dim quadratic feature vectors F(q) and per-Gaussian coefficient
    vectors G (mah) / H (dist^2) so that mah = G.F and dist^2 = H.F.
  - Device (per core, 8192 queries): for each (n-tile 128 x m-chunk 512):
    TensorE: mah/d2 via K=10 matmuls -> PSUM
    ScalarE: amp = Exp(-0.5*mah); pc = Sqrt((f/c)^2 * d2)  [phase in cycles]
    VectorE: Dekker round-to-nearest for range reduction; v = pc - rn(pc)
    ScalarE: sin(2*pi*v), shifted-sin for cos  (HW Sin domain is [-pi,pi])
    VectorE: S = amp*sin, C = amp*cos (fp16)
    TensorE: accumulate [re;im] += [cvr,cvi]^T C + [cvi,-cvr]^T S over n
  - ACT table sets (exp/sqrt/trig) are phase-batched over groups of 4
    m-tiles to amortize the ~2.7us table switch cost.
"""
import numpy as np
from contextlib import ExitStack

import concourse.bass as bass
import concourse.bacc as bacc
import concourse.tile as tile
import concourse.mybir as mybir
from concourse.bass_utils import run_bass_kernel_spmd
from concourse.tile_rust import add_dep_helper

C_LIGHT = 299792458.0
M, N, NCORES = 65536, 1024, 8
MC = M // NCORES            # 8192 queries per core
MT = 512                    # m-tile (free dim per matmul / chunk)
TILES = MC // MT            # 16 m-tiles per core
NT = 128                    # n-tile (partition dim)
NTILES = N // NT            # 8 n-tiles
B = 4                       # m-tiles per table-set superbatch
KSTACK = 40                 # fp16 hi/lo split features: [Fh;Fl;Fh;Fl] x [Gh;Gh;Gl;Gl]
MAGIC = float(1.5 * 2.0 ** 23)
TWO_PI = float(2.0 * np.pi)

F32 = mybir.dt.float32
F16 = mybir.dt.float16
AF = mybir.ActivationFunctionType
ALU = mybir.AluOpType

_cache = {}


def _build(s2: float):
    """Build + compile the per-core Bass program. s2 = (f/c)^2.

    Structure per superbatch (m_b = WB = 2048 queries, 4 per core), phased so
    each ACT table set (exp / sqrt / trig) loads once per superbatch:
      P1 [exp set]: K=40 fp16 matmuls; Exp 1024-wide from PSUM -> amp fp16;
                    d2 drained PSUM->SBUF via DMA, clamped in-place on GPSIMD
      P2 [sqrt set]: Sqrt 2048-wide -> phase in carrier cycles (fp32)
      P3 [trig set]: Dekker rn (DVE) ; v = pc - rn(pc) (GPSIMD) ;
                     cos-path reduction (DVE scalar_tensor_tensor) ;
                     Sin 2048-wide x2 -> fp16 ; amp muls (DVE) ;
                     cv-weighted reduction matmuls into bank-packed [8,512]
                     PSUM accumulators (rows 2c:2c+2 = chunk c's re/im)
    """
    nc = bacc.Bacc("TRN2", target_bir_lowering=False, debug=False,
                   num_devices=NCORES)

    feat_d = nc.dram_tensor("feat", [KSTACK, MC], F16, kind="ExternalInput")
    wg_d = nc.dram_tensor("wg", [TILES, KSTACK, N], F16, kind="ExternalInput")
    wh_d = nc.dram_tensor("wh", [TILES, KSTACK, N], F16, kind="ExternalInput")
    cw1_d = nc.dram_tensor("cw1", [NT, 2 * NTILES], F16, kind="ExternalInput")
    cw2_d = nc.dram_tensor("cw2", [NT, 2 * NTILES], F16, kind="ExternalInput")
    out_d = nc.dram_tensor("out", [2, MC], F32, kind="ExternalOutput")

    WB = 2048                  # superbatch width (queries)
    NSB = MC // WB             # 4 superbatches per core
    CH = WB // MT              # 4 chunks of 512 per superbatch

    with tile.TileContext(nc) as tc, ExitStack() as ctx:
        p_feat = ctx.enter_context(tc.tile_pool(name="feat", bufs=1))
        p_w = ctx.enter_context(tc.tile_pool(name="w", bufs=CH + 1))
        p_cw = ctx.enter_context(tc.tile_pool(name="cw", bufs=1))
        p_amp = ctx.enter_context(tc.tile_pool(name="amp", bufs=NTILES))
        p_dsq = ctx.enter_context(tc.tile_pool(name="dsq", bufs=NTILES + 1))
        p_v = ctx.enter_context(tc.tile_pool(name="v", bufs=4))
        p_trig = ctx.enter_context(tc.tile_pool(name="trig", bufs=3))
        p_stage = ctx.enter_context(tc.tile_pool(name="stage", bufs=1))
        p_mah = ctx.enter_context(tc.tile_pool(name="mah", bufs=2, space="PSUM"))
        p_d2 = ctx.enter_context(tc.tile_pool(name="d2", bufs=2, space="PSUM"))
        p_reim = ctx.enter_context(tc.tile_pool(name="reim", bufs=1, space="PSUM"))

        cw1_t = p_cw.tile([NT, 2 * NTILES], F16, tag="cw1")
        cw2_t = p_cw.tile([NT, 2 * NTILES], F16, tag="cw2")
        nc.sync.dma_start(cw1_t[:], cw1_d[:])
        nc.sync.dma_start(cw2_t[:], cw2_d[:])
        bias_c = p_cw.tile([NT, 1], F32, tag="biasc")
        nc.vector.memset(bias_c[:], float(np.pi / 2 - 2 * np.pi))

        # ACT ops collected per table-set phase; consecutive phases get
        # explicit ordering edges so the scheduler cannot interleave ops of
        # different ACT table sets (each interleave costs a ~2.7us reload).
        act_phases = [[]]

        for sb in range(NSB):
            m0 = sb * WB
            amps, d2s, pcs = {}, {}, {}
            feat_t = p_feat.tile([KSTACK, WB], F16, tag="feat")
            nc.sync.dma_start(feat_t[:], feat_d[:, m0:m0 + WB])
            # ---- P1 [exp set]: matmuls, Exp from PSUM, d2 -> SBUF + clamp --
            for wt in range(2):          # two m-tiles of 1024... actually
                pass
            # m-tiles within superbatch for weight DMAs (W is per 512-tile
            # of the kd ordering: TILES = MC//MT, index ti = m0//MT + c)
            wgs, whs = {}, {}
            for c in range(CH):
                ti = m0 // MT + c
                wg_t = p_w.tile([KSTACK, N], F16, tag="wg")
                wh_t = p_w.tile([KSTACK, N], F16, tag="wh")
                nc.sync.dma_start(wg_t[:], wg_d[ti])
                nc.sync.dma_start(wh_t[:], wh_d[ti])
                wgs[c], whs[c] = wg_t, wh_t
            for t in range(NTILES):
                amp = p_amp.tile([NT, WB], F16, tag="amp")
                for c in range(CH):
                    mah = p_mah.tile([NT, MT], F32, tag="mah")
                    nc.tensor.matmul(mah[:], wgs[c][:, t * NT:(t + 1) * NT],
                                     feat_t[:, c * MT:(c + 1) * MT],
                                     start=True, stop=True)
                    act_phases[-1].append(nc.scalar.activation(
                        amp[:, c * MT:(c + 1) * MT], mah[:], AF.Exp,
                        scale=-0.5))
                amps[t] = amp
                d2sb = p_dsq.tile([NT, WB], F32, tag="dsq")
                for c in range(CH):
                    d2p = p_d2.tile([NT, MT], F32, tag="d2")
                    nc.tensor.matmul(d2p[:], whs[c][:, t * NT:(t + 1) * NT],
                                     feat_t[:, c * MT:(c + 1) * MT],
                                     start=True, stop=True)
                    nc.vector.tensor_scalar(d2sb[:, c * MT:(c + 1) * MT],
                                            d2p[:], 0.0, None, ALU.max)
                d2s[t] = d2sb
            # ---- P2 [sqrt set]: phase in cycles, 2048-wide ----
            vws = {}

            def _reduce_pipe(t):
                pc = pcs[t]
                f_s = p_v.tile([NT, WB], F32, tag="fswc")
                nc.vector.tensor_scalar(f_s[:], pc[:], MAGIC, MAGIC,
                                        ALU.add, ALU.subtract)
                v_s = f_s  # in-place: GPSIMD writes pc - f_s over f_s
                nc.gpsimd.tensor_sub(v_s[:], pc[:], f_s[:])
                w_c = p_v.tile([NT, WB], F32, tag="fswc")
                nc.vector.scalar_tensor_tensor(
                    w_c[:], v_s[:], 0.25, v_s[:], ALU.is_lt, ALU.add)
                vws[t] = (v_s, w_c)

            act_phases.append([])
            for t in range(NTILES):
                pc = p_dsq.tile([NT, WB], F32, tag="dsq")
                act_phases[-1].append(nc.scalar.activation(
                    pc[:], d2s[t][:], AF.Sqrt, scale=float(s2)))
                pcs[t] = pc
                if t < 1:
                    _reduce_pipe(t)
            # ---- P3 [trig set] ----
            act_phases.append([])
            reim = p_reim.tile([2, WB], F32, tag="reim")
            for t in range(NTILES):
                if t >= 1:
                    _reduce_pipe(t)
                v_s, w_c = vws[t]
                amp = amps[t]
                s_t = p_trig.tile([NT, WB], F16, tag="stm")
                act_phases[-1].append(nc.scalar.activation(
                    s_t[:], v_s[:], AF.Sin, scale=TWO_PI))
                c_t = p_trig.tile([NT, WB], F16, tag="ctm")
                act_phases[-1].append(nc.scalar.activation(
                    c_t[:], w_c[:], AF.Sin, scale=TWO_PI, bias=bias_c[:]))
                s_m = p_trig.tile([NT, WB], F16, tag="stm")
                nc.vector.tensor_mul(s_m[:], amp[:], s_t[:])
                c_m = p_trig.tile([NT, WB], F16, tag="ctm")
                nc.vector.tensor_mul(c_m[:], amp[:], c_t[:])
                for c in range(CH):
                    win = reim[:, c * MT:(c + 1) * MT]
                    nc.tensor.matmul(win, cw1_t[:, 2 * t:2 * t + 2],
                                     c_m[:, c * MT:(c + 1) * MT],
                                     start=(t == 0), stop=False)
                    nc.tensor.matmul(win, cw2_t[:, 2 * t:2 * t + 2],
                                     s_m[:, c * MT:(c + 1) * MT],
                                     start=False, stop=(t == NTILES - 1))
            stg = p_stage.tile([2, WB], F32, tag="stg")
            nc.vector.tensor_copy(stg[:], reim[:])
            nc.sync.dma_start(out_d[:, m0:m0 + WB], stg[:])
            act_phases.append([])

        for prev, cur in zip(act_phases, act_phases[1:]):
            if prev and cur:
                add_dep_helper(cur[0].ins, prev[-1].ins, sync=False,
                               reason="ACT table-set phase ordering")

    nc.compile()
    return nc


def _quat_to_rotmat(q):
    q = q / np.linalg.norm(q, axis=-1, keepdims=True)
    w, x, y, z = q[..., 0], q[..., 1], q[..., 2], q[..., 3]
    R = np.stack([
        np.stack([1 - 2 * (y * y + z * z), 2 * (x * y - w * z), 2 * (x * z + w * y)], -1),
        np.stack([2 * (x * y + w * z), 1 - 2 * (x * x + z * z), 2 * (y * z - w * x)], -1),
        np.stack([2 * (x * z - w * y), 2 * (y * z + w * x), 1 - 2 * (x * x + y * y)], -1),
    ], -2)
    return R


def _kd_perm(q, levels=7):
    """Recursive median bisection along the longest axis -> 2^levels
    equal-size spatially compact groups. Returns the permutation."""
    groups = [np.arange(q.shape[0])]
    for _ in range(levels):
        ng = []
        for g in groups:
            ext = q[g].max(0) - q[g].min(0)
            ax = int(np.argmax(ext))
            half = len(g) // 2
            idx = np.argpartition(q[g, ax], half)
            ng.append(g[idx[:half]])
            ng.append(g[idx[half:]])
        groups = ng
    return np.concatenate(groups)


def kernel(query_points, positions, cv_real, cv_imag, scales, rotations,
           frequency):
    q = np.asarray(query_points, np.float64)
    p = np.asarray(positions, np.float64)
    cvr = np.asarray(cv_real, np.float64)
    cvi = np.asarray(cv_imag, np.float64)
    sc = np.asarray(scales, np.float64)
    rot = np.asarray(rotations, np.float64)
    f = float(np.asarray(frequency).item())
    s1 = f / C_LIGHT              # cycles per meter
    s2 = s1 * s1

    # Spatially cluster query points so per-tile centering keeps feature
    # magnitudes small (controls fp32 cancellation error in mah/dist^2)
    perm = _kd_perm(q, levels=7)
    qs = q[perm]

    ntiles_tot = M // MT
    qt = qs.reshape(ntiles_tot, MT, 3)
    c = qt.mean(axis=1)                         # (ntiles, 3) tile centers
    qc = qt - c[:, None, :]                     # centered queries

    # features [ntiles, 10, MT]
    x0, x1, x2 = qc[..., 0], qc[..., 1], qc[..., 2]
    F = np.stack([x0 * x0, x1 * x1, x2 * x2, x0 * x1, x0 * x2, x1 * x2,
                  x0, x1, x2, np.ones_like(x0)], axis=1)

    def _split16(X):
        Xh = X.astype(np.float16)
        Xl = (X - Xh.astype(np.float64)).astype(np.float16)
        return Xh, Xl

    R = _quat_to_rotmat(rot)                    # (N,3,3)
    A = np.einsum("nij,nj,nkj->nik", R, 1.0 / sc ** 2, R)   # (N,3,3)

    pc = p[None, :, :] - c[:, None, :]          # (ntiles, N, 3)
    Ap = np.einsum("nij,tnj->tni", A, pc)       # (ntiles, N, 3)
    G = np.empty((ntiles_tot, 10, N))
    G[:, 0] = A[None, :, 0, 0]; G[:, 1] = A[None, :, 1, 1]; G[:, 2] = A[None, :, 2, 2]
    G[:, 3] = 2 * A[None, :, 0, 1]; G[:, 4] = 2 * A[None, :, 0, 2]; G[:, 5] = 2 * A[None, :, 1, 2]
    G[:, 6] = -2 * Ap[..., 0]; G[:, 7] = -2 * Ap[..., 1]; G[:, 8] = -2 * Ap[..., 2]
    G[:, 9] = np.einsum("tni,tni->tn", pc, Ap)
    H = np.zeros((ntiles_tot, 10, N))
    H[:, 0] = 1.0; H[:, 1] = 1.0; H[:, 2] = 1.0
    H[:, 6] = -2 * pc[..., 0]; H[:, 7] = -2 * pc[..., 1]; H[:, 8] = -2 * pc[..., 2]
    H[:, 9] = np.einsum("tni,tni->tn", pc, pc)

    # reduction weights per n-tile: [cvr|cvi] and [cvi|-cvr] columns
    cw1 = np.empty((NT, 2 * NTILES), np.float16)
    cw2 = np.empty((NT, 2 * NTILES), np.float16)
    for t in range(NTILES):
        cw1[:, 2 * t] = cvr[t * NT:(t + 1) * NT]
        cw1[:, 2 * t + 1] = cvi[t * NT:(t + 1) * NT]
        cw2[:, 2 * t] = cvi[t * NT:(t + 1) * NT]
        cw2[:, 2 * t + 1] = -cvr[t * NT:(t + 1) * NT]

    # fp16 hi/lo stacks: G.F = Gh.Fh + Gh.Fl + Gl.Fh + Gl.Fl  (exact products)
    Fh, Fl = _split16(F)
    Fs = np.concatenate([Fh, Fl, Fh, Fl], axis=1)          # (ntiles, 40, MT)
    Gh, Gl = _split16(G)
    Gs = np.concatenate([Gh, Gh, Gl, Gl], axis=1)          # (ntiles, 40, N)
    Hh, Hl = _split16(H)
    Hs = np.concatenate([Hh, Hh, Hl, Hl], axis=1)

    key = round(s2, 9)
    if key not in _cache:
        _cache[key] = _build(s2)
    nc = _cache[key]

    in_maps = []
    for core in range(NCORES):
        t0, t1 = core * TILES, (core + 1) * TILES
        fc = Fs[t0:t1].transpose(1, 0, 2).reshape(KSTACK, MC)
        in_maps.append({
            "feat": np.ascontiguousarray(fc, np.float16),
            "wg": np.ascontiguousarray(Gs[t0:t1], np.float16),
            "wh": np.ascontiguousarray(Hs[t0:t1], np.float16),
            "cw1": cw1, "cw2": cw2,
        })

    res = run_bass_kernel_spmd(nc, in_maps, list(range(NCORES)))

    field_sorted = np.empty(M, np.complex64)
    for core in range(NCORES):
        o = res.results[core]["out"]
        field_sorted[core * MC:(core + 1) * MC] = o[0] + 1j * o[1]
    field = np.empty(M, np.complex64)
    field[perm] = field_sorted
    return field



# revision 6
# speedup vs baseline: 10.0275x; 10.0275x over previous
"""Complex Gaussian splatter field kernel for 8 TRN2 NeuronCores.

Math: field[m] = sum_n exp(-0.5*mah(m,n)) * exp(-i*k*dist(m,n)) * cv[n]
  mah  = |B_n (q-p_n)|^2,  B_n = diag(1/s_n) R_n^T   (Mahalanobis)
  dist = |q-p_n|

Strategy (vs precomputing per-tile features on host): ship only the raw
per-core query slice [3, 8192] f32 plus ~60KB of replicated per-Gaussian
parameters; build everything on device.

Device per core (M/8 = 8192 queries), per superbatch of 1024 queries:
  - split q into fp16 hi/lo -> feature stack X = [qh(3); ql(3); qh(3)]
  - per n-tile (128 Gaussians in partitions) and 512-query chunk:
    TensorE: 6 matmuls (K=9/6 fp16) -> u'_i = Bh.qh + Bh.ql + Bl.qh and
             v'_i = q_i, exact to ~1e-5
    ScalarE: Square activation with per-partition bias folds the
             constants: sq_ui = (u'_i - (B p)_i)^2, sq_vi = (v'_i - p_i)^2
             (square lives in every ACT table set -> no table switches)
    V/G:     mah = sum sq_u, d2 = sum sq_v
    ScalarE: amp = Exp(-0.5 mah);  pc = Sqrt((f/c)^2 d2) = phase in cycles
    V/G:     round-to-nearest (magic add/sub), frac = pc - rn,
             cos-path shift (HW Sin domain is [-pi, pi])
    ScalarE: sin(2*pi*frac), shifted sin for cos -> fp16
    VectorE: S = amp*sin, C = amp*cos
    TensorE: accumulate [re;im] += [cvr,cvi]^T C + [cvi,-cvr]^T S over n
  Exp/Sqrt/Sin ACT ops are phase-ordered per superbatch so each table set
  loads once (~2.7us per switch).

Dispatch: first call goes through bass_utils.run_bass_kernel_spmd
(compile + run + warm). Reruns reuse a cached jitted dispatcher around
the identical compiled program (same bass_exec custom call, same NEFF)
to avoid the ~300ms per-call jit re-trace + executable reload.
"""
import numpy as np
from contextlib import ExitStack

import concourse.bass as bass
import concourse.bacc as bacc
import concourse.tile as tile
import concourse.mybir as mybir
from concourse.bass_utils import run_bass_kernel_spmd
from concourse.tile_rust import add_dep_helper

C_LIGHT = 299792458.0
M, N, NCORES = 65536, 1024, 8
MC = M // NCORES            # 8192 queries per core
NT = 128                    # n-tile (partition dim)
NTILES = N // NT            # 8 n-tiles
WB = 1024                   # superbatch width (queries)
NSB = MC // WB              # 8 superbatches per core
MT = 512                    # chunk width (matmul free dim / PSUM bank)
CH = WB // MT               # 2 chunks per superbatch
K = 9                       # feature rows [qh(3); ql(3); qh(3)]
MAGIC = float(1.5 * 2.0 ** 23)
TWO_PI = float(2.0 * np.pi)

F32 = mybir.dt.float32
F16 = mybir.dt.float16
AF = mybir.ActivationFunctionType
ALU = mybir.AluOpType

_cache = {}


def _build(s2: float):
    """Build + compile the per-core Bass program. s2 = (f/c)^2."""
    nc = bacc.Bacc("TRN2", target_bir_lowering=False, debug=False,
                   num_devices=NCORES)

    qp_d = nc.dram_tensor("qp", [3, MC], F32, kind="ExternalInput")
    wu_d = nc.dram_tensor("wu", [NTILES, K, 3 * NT], F16, kind="ExternalInput")
    wv_d = nc.dram_tensor("wv", [6, 3 * NT], F16, kind="ExternalInput")
    bias_d = nc.dram_tensor("bias", [NT, 6 * NTILES], F32, kind="ExternalInput")
    cw1_d = nc.dram_tensor("cw1", [NT, 2 * NTILES], F16, kind="ExternalInput")
    cw2_d = nc.dram_tensor("cw2", [NT, 2 * NTILES], F16, kind="ExternalInput")
    out_d = nc.dram_tensor("out", [2, MC], F32, kind="ExternalOutput")

    with tile.TileContext(nc) as tc, ExitStack() as ctx:
        p_const = ctx.enter_context(tc.tile_pool(name="const", bufs=1))
        p_q = ctx.enter_context(tc.tile_pool(name="q", bufs=2))
        p_x = ctx.enter_context(tc.tile_pool(name="x", bufs=2))
        p_amp = ctx.enter_context(tc.tile_pool(name="amp", bufs=NTILES + 1))
        p_d2 = ctx.enter_context(tc.tile_pool(name="d2", bufs=NTILES + 1))
        p_sq = ctx.enter_context(tc.tile_pool(name="sq", bufs=8))
        p_mah = ctx.enter_context(tc.tile_pool(name="mah", bufs=2))
        p_v = ctx.enter_context(tc.tile_pool(name="v", bufs=4))
        p_trig = ctx.enter_context(tc.tile_pool(name="trig", bufs=3))
        p_stage = ctx.enter_context(tc.tile_pool(name="stage", bufs=2))
        p_ps = ctx.enter_context(tc.tile_pool(name="ps", bufs=6, space="PSUM"))
        p_reim = ctx.enter_context(tc.tile_pool(name="reim", bufs=1,
                                                space="PSUM"))

        wu_t = p_const.tile([K, NTILES * 3 * NT], F16, tag="wu")
        for t in range(NTILES):
            nc.sync.dma_start(wu_t[:, t * 3 * NT:(t + 1) * 3 * NT], wu_d[t])
        wv_t = p_const.tile([6, 3 * NT], F16, tag="wv")
        nc.sync.dma_start(wv_t[:], wv_d[:])
        bias_t = p_const.tile([NT, 6 * NTILES], F32, tag="bias")
        nc.sync.dma_start(bias_t[:], bias_d[:])
        cw1_t = p_const.tile([NT, 2 * NTILES], F16, tag="cw1")
        cw2_t = p_const.tile([NT, 2 * NTILES], F16, tag="cw2")
        nc.sync.dma_start(cw1_t[:], cw1_d[:])
        nc.sync.dma_start(cw2_t[:], cw2_d[:])
        bias_c = p_const.tile([NT, 1], F32, tag="biasc")
        nc.vector.memset(bias_c[:], float(np.pi / 2 - 2 * np.pi))

        # ACT ops per table-set phase; consecutive phases get ordering edges
        # (Square is in every table set, so squares stay out of the lists).
        act_phases = [[]]

        for sb in range(NSB):
            m0 = sb * WB
            qs = p_q.tile([3, WB], F32, tag="qs")
            nc.sync.dma_start(qs[:], qp_d[:, m0:m0 + WB])
            # compute qh/ql in partition-0-based tiles (compute ops cannot
            # address partition offsets 3/6), assemble X via SBUF DMAs
            qh16 = p_x.tile([3, WB], F16, tag="qh16")
            hi32 = p_x.tile([3, WB], F32, tag="hi32")
            ql16 = p_x.tile([3, WB], F16, tag="ql16")
            nc.vector.tensor_copy(qh16[:], qs[:])           # qh = f16(q)
            nc.vector.tensor_copy(hi32[:], qh16[:])         # back to f32
            nc.vector.tensor_sub(ql16[:], qs[:], hi32[:])   # ql = q - qh
            X = p_x.tile([K, WB], F16, tag="X")
            nc.sync.dma_start(X[0:3], qh16[:])
            nc.sync.dma_start(X[3:6], ql16[:])
            nc.sync.dma_start(X[6:9], qh16[:])

            # ---- P1 [exp set]: matmuls, squares, mah/d2, amp ----
            amps, d2s, pcs = {}, {}, {}
            for t in range(NTILES):
                amp = p_amp.tile([NT, WB], F16, tag="amp")
                d2sb = p_d2.tile([NT, WB], F32, tag="dsq")
                for c in range(CH):
                    Xc = X[:, c * MT:(c + 1) * MT]
                    us, vs = [], []
                    for i in range(3):
                        pu = p_ps.tile([NT, MT], F32, tag="uv")
                        nc.tensor.matmul(
                            pu[:],
                            wu_t[:, (t * 3 + i) * NT:(t * 3 + i + 1) * NT],
                            Xc, start=True, stop=True)
                        us.append(pu)
                    for i in range(3):
                        pv = p_ps.tile([NT, MT], F32, tag="uv")
                        nc.tensor.matmul(
                            pv[:], wv_t[:, i * NT:(i + 1) * NT],
                            X[0:6, c * MT:(c + 1) * MT],
                            start=True, stop=True)
                        vs.append(pv)
                    sq = []
                    for i in range(3):
                        squ = p_sq.tile([NT, MT], F32, tag="sq")
                        nc.scalar.activation(
                            squ[:], us[i][:], AF.Square,
                            bias=bias_t[:, t * 6 + i:t * 6 + i + 1])
                        sq.append(squ)
                    for i in range(3):
                        sqv = p_sq.tile([NT, MT], F32, tag="sq")
                        nc.scalar.activation(
                            sqv[:], vs[i][:], AF.Square,
                            bias=bias_t[:, t * 6 + 3 + i:t * 6 + 3 + i + 1])
                        sq.append(sqv)
                    mah = p_mah.tile([NT, MT], F32, tag="mah")
                    nc.gpsimd.tensor_add(mah[:], sq[0][:], sq[1][:])
                    nc.gpsimd.tensor_add(mah[:], mah[:], sq[2][:])
                    d2c = d2sb[:, c * MT:(c + 1) * MT]
                    nc.vector.tensor_add(d2c, sq[3][:], sq[4][:])
                    nc.vector.tensor_add(d2c, d2c, sq[5][:])
                    act_phases[-1].append(nc.scalar.activation(
                        amp[:, c * MT:(c + 1) * MT], mah[:], AF.Exp,
                        scale=-0.5))
                amps[t] = amp
                d2s[t] = d2sb

            # ---- P2 [sqrt set]: phase in carrier cycles ----
            vws = {}

            def _reduce_pipe(t):
                pc = pcs[t]
                f_s = p_v.tile([NT, WB], F32, tag="fswc")
                nc.vector.tensor_scalar(f_s[:], pc[:], MAGIC, MAGIC,
                                        ALU.add, ALU.subtract)
                v_s = f_s  # in-place: GPSIMD writes pc - f_s over f_s
                nc.gpsimd.tensor_sub(v_s[:], pc[:], f_s[:])
                w_c = p_v.tile([NT, WB], F32, tag="fswc")
                nc.vector.scalar_tensor_tensor(
                    w_c[:], v_s[:], 0.25, v_s[:], ALU.is_lt, ALU.add)
                vws[t] = (v_s, w_c)

            act_phases.append([])
            for t in range(NTILES):
                pc = p_d2.tile([NT, WB], F32, tag="dsq")
                act_phases[-1].append(nc.scalar.activation(
                    pc[:], d2s[t][:], AF.Sqrt, scale=float(s2)))
                pcs[t] = pc
                if t < 1:
                    _reduce_pipe(t)

            # ---- P3 [trig set] ----
            act_phases.append([])
            reim = p_reim.tile([2, WB], F32, tag="reim")
            for t in range(NTILES):
                if t >= 1:
                    _reduce_pipe(t)
                v_s, w_c = vws[t]
                amp = amps[t]
                s_t = p_trig.tile([NT, WB], F16, tag="stm")
                act_phases[-1].append(nc.scalar.activation(
                    s_t[:], v_s[:], AF.Sin, scale=TWO_PI))
                c_t = p_trig.tile([NT, WB], F16, tag="ctm")
                act_phases[-1].append(nc.scalar.activation(
                    c_t[:], w_c[:], AF.Sin, scale=TWO_PI, bias=bias_c[:]))
                s_m = p_trig.tile([NT, WB], F16, tag="stm")
                nc.vector.tensor_mul(s_m[:], amp[:], s_t[:])
                c_m = p_trig.tile([NT, WB], F16, tag="ctm")
                nc.vector.tensor_mul(c_m[:], amp[:], c_t[:])
                for c in range(CH):
                    win = reim[:, c * MT:(c + 1) * MT]
                    nc.tensor.matmul(win, cw1_t[:, 2 * t:2 * t + 2],
                                     c_m[:, c * MT:(c + 1) * MT],
                                     start=(t == 0), stop=False)
                    nc.tensor.matmul(win, cw2_t[:, 2 * t:2 * t + 2],
                                     s_m[:, c * MT:(c + 1) * MT],
                                     start=False, stop=(t == NTILES - 1))
            stg = p_stage.tile([2, WB], F32, tag="stg")
            nc.vector.tensor_copy(stg[:], reim[:])
            nc.sync.dma_start(out_d[:, m0:m0 + WB], stg[:])
            act_phases.append([])

        for prev, cur in zip(act_phases, act_phases[1:]):
            if prev and cur:
                add_dep_helper(cur[0].ins, prev[-1].ins, sync=False,
                               reason="ACT table-set phase ordering")

    nc.compile()
    return nc


class _CachedRunner:
    """Re-dispatch the compiled Bass program without re-tracing jax.jit.

    Mirrors concourse.bass2jax.run_bass_via_pjrt's multi-core path, but the
    jitted shard_map callable (and therefore the loaded PJRT executable) is
    built once and reused; each call still re-uploads all inputs and runs
    the kernel on the 8 cores.
    """

    def __init__(self, nc):
        import jax
        from jax.sharding import Mesh, PartitionSpec
        from jax.experimental.shard_map import shard_map
        from concourse.bass2jax import (_bass_exec_p, install_neuronx_cc_hook,
                                        partition_id_tensor)

        install_neuronx_cc_hook()
        assert not nc.dbg_callbacks
        self.dbg_name = None
        if nc.dbg_addr is not None:
            # unused ExternalInput; bind zeros (uint32[1,2], see bass2jax)
            self.dbg_name = nc.dbg_addr.name
        partition_name = (nc.partition_id_tensor.name
                          if nc.partition_id_tensor else None)
        self.in_names, self.out_names = [], []
        out_avals, zero_outs = [], []
        for alloc in nc.m.functions[0].allocations:
            if not isinstance(alloc, mybir.MemoryLocationSet):
                continue
            name = alloc.memorylocations[0].name
            if alloc.kind == "ExternalInput":
                if name != partition_name:
                    self.in_names.append(name)
            elif alloc.kind == "ExternalOutput":
                shape = tuple(alloc.tensor_shape)
                dtype = mybir.dt.np(alloc.dtype)
                self.out_names.append(name)
                out_avals.append(jax.core.ShapedArray(shape, dtype))
                zero_outs.append(
                    np.zeros((NCORES * shape[0], *shape[1:]), dtype))
        self.out_shapes = [a.shape for a in out_avals]
        self.zero_outs = zero_outs
        n_params = len(self.in_names)
        all_names = (tuple(self.in_names) + tuple(self.out_names)
                     + ((partition_name,) if partition_name else ()))
        out_names = tuple(self.out_names)
        out_avals_t = tuple(out_avals)

        def _body(*args):
            operands = list(args)
            if partition_name is not None:
                operands.append(partition_id_tensor())
            outs = _bass_exec_p.bind(
                *operands, out_avals=out_avals_t, in_names=all_names,
                out_names=out_names, lowering_input_output_aliases=(),
                sim_require_finite=True, sim_require_nnan=True, nc=nc)
            return tuple(outs)

        devices = jax.devices()[:NCORES]
        mesh = Mesh(np.asarray(devices), ("core",))
        n_outs = len(out_names)
        self._fn = jax.jit(
            shard_map(_body, mesh=mesh,
                      in_specs=(PartitionSpec("core"),) * (n_params + n_outs),
                      out_specs=(PartitionSpec("core"),) * n_outs,
                      check_rep=False),
            donate_argnums=tuple(range(n_params, n_params + n_outs)),
            keep_unused=True)

    def __call__(self, in_maps):
        if self.dbg_name is not None:
            z = np.zeros((1, 2), np.uint32)
            in_maps = [{**m, self.dbg_name: z} for m in in_maps]
        concat_in = [
            np.concatenate([np.asarray(m[name]) for m in in_maps], axis=0)
            for name in self.in_names
        ]
        out_arrs = self._fn(*concat_in, *self.zero_outs)
        return [
            {name: np.asarray(out_arrs[i]).reshape(
                NCORES, *self.out_shapes[i])[c]
             for i, name in enumerate(self.out_names)}
            for c in range(NCORES)
        ]


def _quat_to_rotmat(q):
    q = q / np.linalg.norm(q, axis=-1, keepdims=True)
    w, x, y, z = q[..., 0], q[..., 1], q[..., 2], q[..., 3]
    R = np.stack([
        np.stack([1 - 2 * (y * y + z * z), 2 * (x * y - w * z), 2 * (x * z + w * y)], -1),
        np.stack([2 * (x * y + w * z), 1 - 2 * (x * x + z * z), 2 * (y * z - w * x)], -1),
        np.stack([2 * (x * z - w * y), 2 * (y * z + w * x), 1 - 2 * (x * x + y * y)], -1),
    ], -2)
    return R


def _split16(X):
    Xh = X.astype(np.float16)
    Xl = (X - Xh.astype(np.float64)).astype(np.float16)
    return Xh, Xl


def kernel(query_points, positions, cv_real, cv_imag, scales, rotations,
           frequency):
    q = np.asarray(query_points, np.float64)
    p = np.asarray(positions, np.float64)
    cvr = np.asarray(cv_real, np.float64)
    cvi = np.asarray(cv_imag, np.float64)
    sc = np.asarray(scales, np.float64)
    rot = np.asarray(rotations, np.float64)
    f = float(np.asarray(frequency).item())
    s1 = f / C_LIGHT
    s2 = s1 * s1

    R = _quat_to_rotmat(rot)                     # (N,3,3)
    B = R.transpose(0, 2, 1) / sc[:, :, None]    # diag(1/s) R^T, (N,3,3)
    Bh, Bl = _split16(B)
    # wu[t, 0:3, i*NT+nl] = Bh[n,i,:], rows 3:6 same (pairs with ql),
    # rows 6:9 = Bl (pairs with qh);  n = t*NT + nl
    Bh_r = Bh.reshape(NTILES, NT, 3, 3).transpose(0, 3, 2, 1).reshape(
        NTILES, 3, 3 * NT)
    Bl_r = Bl.reshape(NTILES, NT, 3, 3).transpose(0, 3, 2, 1).reshape(
        NTILES, 3, 3 * NT)
    wu = np.concatenate([Bh_r, Bh_r, Bl_r], axis=1).astype(np.float16)
    wu = np.ascontiguousarray(wu)

    # v_i = q_i (rows {i, i+3} are ones), constant across n-tiles
    wv = np.zeros((6, 3 * NT), np.float16)
    for i in range(3):
        wv[i, i * NT:(i + 1) * NT] = 1.0
        wv[i + 3, i * NT:(i + 1) * NT] = 1.0

    # per-partition f32 biases: cols t*6+i = -(B p)_i, t*6+3+i = -p_i
    Bp = np.einsum("nij,nj->ni", B, p)           # (N,3)
    bias_arr = np.empty((NTILES, NT, 6), np.float32)
    bias_arr[:, :, 0:3] = -Bp.reshape(NTILES, NT, 3)
    bias_arr[:, :, 3:6] = -p.reshape(NTILES, NT, 3)
    bias_arr = np.ascontiguousarray(
        bias_arr.transpose(1, 0, 2).reshape(NT, 6 * NTILES))

    cw1 = np.empty((NT, 2 * NTILES), np.float16)
    cw2 = np.empty((NT, 2 * NTILES), np.float16)
    for t in range(NTILES):
        cw1[:, 2 * t] = cvr[t * NT:(t + 1) * NT]
        cw1[:, 2 * t + 1] = cvi[t * NT:(t + 1) * NT]
        cw2[:, 2 * t] = cvi[t * NT:(t + 1) * NT]
        cw2[:, 2 * t + 1] = -cvr[t * NT:(t + 1) * NT]

    qT = np.ascontiguousarray(q.T.astype(np.float32))  # [3, M]

    key = round(s2, 9)
    if key not in _cache:
        _cache[key] = {"nc": _build(s2), "runner": None}
    st = _cache[key]
    nc = st["nc"]

    in_maps = []
    for core in range(NCORES):
        in_maps.append({
            "qp": qT[:, core * MC:(core + 1) * MC],
            "wu": wu, "wv": wv, "bias": bias_arr,
            "cw1": cw1, "cw2": cw2,
        })

    if st["runner"] is None:
        res = run_bass_kernel_spmd(nc, in_maps, list(range(NCORES)))
        results = res.results
        runner = _CachedRunner(nc)
        warm = runner(in_maps)  # trace+compile+load once, off the hot path
        for c in range(NCORES):
            np.testing.assert_allclose(warm[c]["out"], results[c]["out"],
                                       rtol=0, atol=1e-5)
        st["runner"] = runner
    else:
        results = st["runner"](in_maps)

    field = np.empty(M, np.complex64)
    for core in range(NCORES):
        o = results[core]["out"]
        field[core * MC:(core + 1) * MC] = o[0] + 1j * o[1]
    return field
